# revision 1
# baseline (speedup 1.0000x reference)
"""BranchingAttention (ViewFormer) Trainium2 Bass kernel.

Problem: two token streams x0 (trunk) / x1, fused qkv projection
(w_attn packs v|q|k), block-causal multi-end attention:
  query token t in block i of branch e attends ALL tokens of trunk
  blocks j < i plus causally (u <= t) its own branch's block i,
joint softmax, out projection.  Returns (out0, out1).

Sharding (8 cores): data-parallel over batch (B=2) x tensor-parallel
over 4 head-groups of 3 heads.  Each core computes BOTH branches for
its 3 heads and emits partial projections; the host sums the 4 head
-group partials per (branch, batch) and adds b_proj.

Per-core device kernel (uniform SPMD program, fp32r matmuls):
  - inputs are host-pre-transposed (xT: [d, tok]) and head-sliced
  - qkv: psum[grp] = sum_dchunk Wg[dc].T @ xT[dc] -> QV sbuf
    groups g0=[q0|k2] g1=[q1|v0] g2=[q2|v1] g3=[k0|v2] g4=[k1|-]
    so all q/k land at partition base 0 (k2 realigned via sbuf dma)
    and all v at base 64 (transposed with a base-64 stacked identity).
  - v natural layout via PE transposes + ones column -> AV lhsT [128,65]
  - scoresT chunks: lhsT = kT[64, 128keys], rhs = qT[64, qspan],
    psum regions [128,<=1024], one ACT exp (scale=1/8 folded) per region
    writing fp32r expT; causal corners fixed post-exp (DVE 0/1-mask mult,
    gpsimd memset for the fully-masked corner).
  - AV: O[0:65, q] += [v|1].T @ expT (denominator rides along as row 64),
    banks pre-zeroed by a K=1 zero matmul so accumulation bits are sane.
  - softmax normalize: ACT copies denom row -> [1,1280], K=1 ones matmul
    broadcasts it to O[64:128], DVE reciprocal -> sbuf, DVE mult -> aT.
  - proj: out[tok,768] = aT(2 k-chunks: 128+64).T @ Wp rows, psum -> sbuf
    -> DRAM partials.
"""
import sys

sys.path.insert(0, "/opt/trn_rl_repo")

import numpy as np

import concourse.bacc as bacc
import concourse.mybir as mybir
import concourse.tile as tile
from concourse.bass_utils import run_bass_kernel_spmd

F32 = mybir.dt.float32
F32R = mybir.dt.float32r

B, NB, BS = 2, 20, 64
D, H = 768, 12
DH = D // H                      # 64
T = NB * BS                      # 1280 tokens per (batch, branch)
DC = D // 128                    # 6 d-chunks
NG = 5                           # qkv col groups of 128
NP = NB // 2                     # 10 key-chunk pairs
H3 = 3                           # heads per core
REG_W = 1024                     # scoresT psum region width
SCALE = 1.0 / np.sqrt(DH)


def round_f32r(x: np.ndarray) -> np.ndarray:
    """Round fp32 to the fp32r grid (11-bit mantissa, RNE)."""
    u = np.ascontiguousarray(x, dtype=np.float32).view(np.uint32).astype(np.uint64)
    u = (u + 0x800 + ((u >> 12) & 1)) & 0xFFFFF000
    return u.astype(np.uint32).view(np.float32)


# ---------------------------------------------------------------- device IR


def _chunk_plan(e):
    """Ordered (kind, idx, qoff, width) score-chunk segments for branch e.

    kind: 's' self (keys = k_e pair idx), 't' trunk (keys = k0 pair idx).
    Query span is [qoff, T).  Order matters: AV accumulation requires the
    zero-filled O banks, self first for e=1 so every column is covered.
    """
    segs = []
    if e == 1:
        for p in range(NP):
            segs.append(("s", p, 128 * p, 128))
        for c in range(NP):
            segs.append(("t", c, 128 * c + 64, T - 128 * c - 64))
    else:
        for c in range(NP):
            segs.append(("t", c, 128 * c, T - 128 * c))
    return segs


def build():
    nc = bacc.Bacc()

    xt = nc.dram_tensor("xt", [DC, 128, T], F32R, kind="ExternalInput")
    xs = nc.dram_tensor("xs", [DC, 128, T], F32R, kind="ExternalInput")
    wq = nc.dram_tensor("wq", [DC, 128, NG * 128], F32R, kind="ExternalInput")
    bq = nc.dram_tensor("bq", [128, NG], F32, kind="ExternalInput")
    wp = nc.dram_tensor("wp", [2, 128, D], F32R, kind="ExternalInput")
    i2 = nc.dram_tensor("i2", [128, 64], F32R, kind="ExternalInput")
    on = nc.dram_tensor("on", [1, 512], F32R, kind="ExternalInput")
    zv = nc.dram_tensor("zv", [1, 65], F32R, kind="ExternalInput")
    m0 = nc.dram_tensor("m0", [128, 128], F32R, kind="ExternalInput")
    m1 = nc.dram_tensor("m1", [128, 64], F32R, kind="ExternalInput")
    ms = nc.dram_tensor("ms", [128, 128], F32R, kind="ExternalInput")
    vo = nc.dram_tensor("vo", [128, NP, H3, 1], F32R, kind="ExternalInput")
    o0 = nc.dram_tensor("o0", [T, D], F32, kind="ExternalOutput")
    o1 = nc.dram_tensor("o1", [T, D], F32, kind="ExternalOutput")
    outs = (o0, o1)

    with tile.TileContext(nc) as tc:
        with (
            tc.tile_pool(name="consts", bufs=1) as cp,
            tc.tile_pool(name="big", bufs=1) as bp,
            tc.tile_pool(name="xtp", bufs=DC) as xtp,
            tc.tile_pool(name="expp", bufs=2) as expp,
            tc.tile_pool(name="outst", bufs=3) as outst,
            tc.tile_pool(name="rrecp", bufs=2) as rrecp,
            tc.tile_pool(name="rbp", bufs=2) as rbp,
            tc.tile_pool(name="scrp", bufs=2, space="DRAM") as scrp,
            tc.tile_pool(name="work", bufs=2, space="PSUM") as work,
            tc.tile_pool(name="psO", bufs=1, space="PSUM") as psO,
        ):
            # ---- constants
            wq_sb = cp.tile([128, DC, NG * 128], F32R)
            nc.sync.dma_start(wq_sb[:], wq[:].rearrange("c p f -> p c f"))
            bq_sb = cp.tile([128, NG], F32)
            nc.sync.dma_start(bq_sb[:], bq[:])
            wp_sb = cp.tile([128, 2, D], F32R)
            nc.sync.dma_start(wp_sb[:], wp[:].rearrange("c p f -> p c f"))
            i2_sb = cp.tile([128, 64], F32R)
            nc.sync.dma_start(i2_sb[:], i2[:])
            on_sb = cp.tile([1, 512], F32R)
            nc.sync.dma_start(on_sb[:], on[:])
            zv_sb = cp.tile([1, 65], F32R)
            nc.sync.dma_start(zv_sb[:], zv[:])
            m0_sb = cp.tile([128, 128], F32R)
            nc.sync.dma_start(m0_sb[:], m0[:])
            m1_sb = cp.tile([128, 64], F32R)
            nc.sync.dma_start(m1_sb[:], m1[:])
            ms_sb = cp.tile([128, 128], F32R)
            nc.sync.dma_start(ms_sb[:], ms[:])

            # ---- persistent per-source tensors
            QV, K2, VNA = [], [], []
            for s, xdram in ((0, xt), (1, xs)):
                qv = bp.tile([128, NG, T], F32R, name=f"qv{s}")
                k2 = bp.tile([64, T], F32R, name=f"k2{s}")
                vna = bp.tile([128, NP, H3, 65], F32R, name=f"vna{s}")
                nc.sync.dma_start(vna[:, :, :, 64:65], vo[:])
                QV.append(qv)
                K2.append(k2)
                VNA.append(vna)
            aT01 = [bp.tile([128, T], F32R, name=f"a01_{e}") for e in range(2)]
            aT2 = [bp.tile([64, T], F32R, name=f"a2_{e}") for e in range(2)]

            # views --------------------------------------------------------
            def qT(s, h):
                return QV[s][0:64, h, :]

            def kT(s, h):
                return (QV[s][0:64, 3, :], QV[s][0:64, 4, :], K2[s][:, :])[h]

            def vT(s, h):  # partition base 64
                return QV[s][64:128, 1 + h, :]

            # ---- phase 1: qkv projections -> QV
            for s, xdram in ((0, xt), (1, xs)):
                xtiles = []
                for dc in range(DC):
                    xtile = xtp.tile([128, T], F32R, tag="xt")
                    nc.sync.dma_start(xtile[:], xdram[dc])
                    xtiles.append(xtile)
                for g in range(NG):
                    for lo, w in ((0, 512), (512, 512), (1024, 256)):
                        pg = work.tile([128, REG_W], F32, tag="work")
                        for dc in range(DC):
                            nc.tensor.matmul(
                                pg[:, 0:w],
                                wq_sb[:, dc, 128 * g : 128 * (g + 1)],
                                xtiles[dc][:, lo : lo + w],
                                start=(dc == 0),
                                stop=(dc == DC - 1),
                            )
                        nc.vector.tensor_scalar_add(
                            QV[s][:, g, lo : lo + w], pg[:, 0:w], bq_sb[:, g : g + 1]
                        )
                # realign k2 (group 0 high half) to partition base 0
                nc.sync.dma_start(K2[s][:], QV[s][64:128, 0, :])

                # ---- phase 2: v natural layout (+ones col already DMA'd)
                for h in range(H3):
                    pt = work.tile([128, REG_W], F32R, tag="work")
                    for tch in range(NP):
                        nc.tensor.transpose(
                            pt[:, 64 * tch : 64 * (tch + 1)],
                            vT(s, h)[:, 128 * tch : 128 * (tch + 1)],
                            i2_sb[64:128, :],
                        )
                    nc.vector.tensor_copy(
                        VNA[s][:, :, h, 0:64],
                        pt[:, 0 : 64 * NP].rearrange("p (tc d) -> p tc d", d=64),
                    )

            # ---- phase 3: attention per (branch, head)
            for e in range(2):
                sq = 0 if e == 0 else 1
                for h in range(H3):
                    O = psO.tile([128, 1280], F32, tag="O")
                    # zero-fill rows 0:65 of all O banks (sets has_written)
                    for lo, w in ((0, 512), (512, 512), (1024, 256)):
                        nc.tensor.matmul(
                            O[0:65, lo : lo + w],
                            zv_sb[:],
                            on_sb[:, 0:w],
                            start=True,
                            stop=False,
                            skip_group_check=True,
                        )

                    # pack score segments into psum regions
                    segs = _chunk_plan(e)
                    regions = []  # list of (rtile_parts,) each part: (kind, idx, qoff, loc, w)
                    cur, used = [], 0
                    for kind, idx, qoff, width in segs:
                        off = 0
                        while off < width:
                            if REG_W - used < 256:
                                regions.append((cur, used))
                                cur, used = [], 0
                            w = min(width - off, REG_W - used)
                            cur.append((kind, idx, qoff + off, used, w, off))
                            used += w
                            off += w
                    if cur:
                        regions.append((cur, used))

                    for parts, used in regions:
                        rt = work.tile([128, REG_W], F32, tag="work")
                        et = expp.tile([128, REG_W], F32R, tag="expT")
                        for kind, idx, qo, loc, w, choff in parts:
                            kv = kT(sq, h) if kind == "s" else kT(0, h)
                            lhsT = kv[:, 128 * idx : 128 * (idx + 1)]
                            rhs = qT(sq, h)[:, qo : qo + w]
                            # split at psum bank boundaries (512 within rt)
                            p0 = 0
                            while p0 < w:
                                bw = min(w - p0, 512 - ((loc + p0) % 512))
                                nc.tensor.matmul(
                                    rt[:, loc + p0 : loc + p0 + bw],
                                    lhsT,
                                    rhs[:, p0 : p0 + bw],
                                    start=True,
                                    stop=True,
                                )
                                p0 += bw
                        nc.scalar.activation(
                            et[:, 0:used],
                            rt[:, 0:used],
                            mybir.ActivationFunctionType.Exp,
                            bias=0.0,
                            scale=float(SCALE),
                        )
                        # causal corner fixes (chunk-local cols 0:128 / 0:64)
                        for kind, idx, qo, loc, w, choff in parts:
                            if kind == "s":
                                if choff < 128:
                                    cw = min(128 - choff, w)
                                    nc.vector.tensor_tensor(
                                        et[:, loc : loc + cw],
                                        et[:, loc : loc + cw],
                                        ms_sb[:, choff : choff + cw],
                                        mybir.AluOpType.mult,
                                    )
                            elif e == 0:
                                if choff < 128:
                                    cw = min(128 - choff, w)
                                    nc.vector.tensor_tensor(
                                        et[:, loc : loc + cw],
                                        et[:, loc : loc + cw],
                                        m0_sb[:, choff : choff + cw],
                                        mybir.AluOpType.mult,
                                    )
                            else:
                                if choff < 64:
                                    cw = min(64 - choff, w)
                                    nc.vector.tensor_tensor(
                                        et[:, loc : loc + cw],
                                        et[:, loc : loc + cw],
                                        m1_sb[:, choff : choff + cw],
                                        mybir.AluOpType.mult,
                                    )
                        # AV accumulate into O
                        for kind, idx, qo, loc, w, choff in parts:
                            vsrc = VNA[sq] if kind == "s" else VNA[0]
                            lhsT = vsrc[:, idx, h, :]
                            p0 = 0
                            while p0 < w:
                                q0 = qo + p0
                                bw = min(w - p0, 512 - (q0 % 512))
                                nc.tensor.matmul(
                                    O[0:65, q0 : q0 + bw],
                                    lhsT,
                                    et[:, loc + p0 : loc + p0 + bw],
                                    start=False,
                                    stop=False,
                                    skip_group_check=True,
                                )
                                p0 += bw

                    # normalize: recip of denom row -> DRAM -> broadcast DMA
                    rrec = rrecp.tile([1, T], F32, tag="rrec")
                    nc.vector.reciprocal(rrec[:], O[64:65, :])
                    scr = scrp.tile([1, T], F32, tag="scr")
                    nc.sync.dma_start(scr[:], rrec[:])
                    rb = rbp.tile([64, T], F32, tag="rb")
                    nc.sync.dma_start(rb[:], scr[:].to_broadcast([64, T]))
                    target = (aT01[e][0:64, :], aT01[e][64:128, :], aT2[e][:, :])[h]
                    nc.vector.tensor_tensor(
                        target, O[0:64, :], rb[:], mybir.AluOpType.mult
                    )

            # ---- phase 4: partial projections
            for e in range(2):
                for m in range(NP):
                    pp = work.tile([128, REG_W], F32, tag="work")
                    for lo, w in ((0, 512), (512, 256)):
                        nc.tensor.matmul(
                            pp[:, lo : lo + w],
                            aT01[e][:, 128 * m : 128 * (m + 1)],
                            wp_sb[:, 0, lo : lo + w],
                            start=True,
                            stop=False,
                        )
                        nc.tensor.matmul(
                            pp[:, lo : lo + w],
                            aT2[e][:, 128 * m : 128 * (m + 1)],
                            wp_sb[0:64, 1, lo : lo + w],
                            start=False,
                            stop=True,
                        )
                    ot = outst.tile([128, D], F32, tag="ot")
                    nc.vector.tensor_copy(ot[:], pp[:, 0:D])
                    nc.sync.dma_start(
                        outs[e][128 * m : 128 * (m + 1), :], ot[:]
                    )

    nc.finalize()
    return nc


# ---------------------------------------------------------------- host side

_NC = None


def _get_nc():
    global _NC
    if _NC is None:
        _NC = build()
    return _NC


def _consts():
    i2 = np.zeros((128, 64), np.float32)
    i2[:64] = np.eye(64, dtype=np.float32)
    i2[64:] = np.eye(64, dtype=np.float32)
    on = np.ones((1, 512), np.float32)
    zv = np.zeros((1, 65), np.float32)
    p = np.arange(128)[:, None]
    x = np.arange(128)[None, :]
    m0 = (p <= x).astype(np.float32)
    ms = np.where(x < 64, p <= x, (p >= 64) & (p <= x)).astype(np.float32)
    vo = np.ones((128, NP, H3, 1), np.float32)
    m1 = np.zeros((128, 64), np.float32)
    m1[0:64] = 1.0
    return dict(i2=i2, on=on, zv=zv, m0=m0, ms=ms, m1=m1, vo=vo)


def _core_inputs(x0, x1, w_attn, b_attn, w_proj, consts):
    """Build the 8 per-core input maps. Core order: (b, G) row-major."""
    maps = []
    xT = [
        [round_f32r(x[b].reshape(T, D).T).reshape(DC, 128, T) for b in range(B)]
        for x in (x0, x1)
    ]
    for b in range(B):
        for G in range(4):
            gh = [3 * G + h for h in range(H3)]
            qc = [768 + g * 64 + np.arange(64) for g in gh]
            kc = [1536 + g * 64 + np.arange(64) for g in gh]
            vc = [0 + g * 64 + np.arange(64) for g in gh]
            groups = [
                np.concatenate([qc[0], kc[2]]),
                np.concatenate([qc[1], vc[0]]),
                np.concatenate([qc[2], vc[1]]),
                np.concatenate([kc[0], vc[2]]),
                np.concatenate([kc[1], kc[1]]),  # pad half unused
            ]
            cols = np.concatenate(groups)
            wqm = w_attn[:, cols].copy()
            wqm[:, 4 * 128 + 64 :] = 0.0
            bqm = b_attn[cols].reshape(NG, 128).T.copy()
            bqm[64:, 4] = 0.0
            wpm = np.zeros((2, 128, D), np.float32)
            wpm[0] = w_proj[3 * G * 64 : 3 * G * 64 + 128]
            wpm[1, 0:64] = w_proj[3 * G * 64 + 128 : 3 * G * 64 + 192]
            maps.append(
                dict(
                    xt=xT[0][b],
                    xs=xT[1][b],
                    wq=round_f32r(wqm).reshape(DC, 128, NG * 128),
                    bq=np.ascontiguousarray(bqm, np.float32),
                    wp=round_f32r(wpm),
                    **consts,
                )
            )
    return maps


def kernel(x0, x1, w_attn, b_attn, w_proj, b_proj, _trace=False):
    x0 = np.asarray(x0, np.float32)
    x1 = np.asarray(x1, np.float32)
    w_attn = np.asarray(w_attn, np.float32)
    b_attn = np.asarray(b_attn, np.float32)
    w_proj = np.asarray(w_proj, np.float32)
    b_proj = np.asarray(b_proj, np.float32)

    nc = _get_nc()
    maps = _core_inputs(x0, x1, w_attn, b_attn, w_proj, _consts())
    res = run_bass_kernel_spmd(nc, maps, core_ids=list(range(8)), trace=_trace)

    out = [np.zeros((B, T, D), np.float32) for _ in range(2)]
    for ci, r in enumerate(res.results):
        b = ci // 4
        out[0][b] += r["o0"]
        out[1][b] += r["o1"]
    out0 = (out[0] + b_proj).reshape(B, NB, BS, D)
    out1 = (out[1] + b_proj).reshape(B, NB, BS, D)
    if _trace:
        kernel._last = res
    return out0, out1


if __name__ == "__main__":
    rng = np.random.default_rng(0)
    x0 = rng.standard_normal((B, NB, BS, D), dtype=np.float32)
    x1 = rng.standard_normal((B, NB, BS, D), dtype=np.float32)
    wa = rng.standard_normal((D, 3 * D), dtype=np.float32) * 0.02
    ba = np.zeros(3 * D, np.float32)
    wpj = rng.standard_normal((D, D), dtype=np.float32) * 0.02
    bp_ = np.zeros(D, np.float32)
    o0, o1 = kernel(x0, x1, wa, ba, wpj, bp_)
    print("ran", o0.shape, o1.shape, float(np.abs(o0).mean()))

